# revision 37
# baseline (speedup 1.0000x reference)
"""Grouped-Query Attention (B=2, S=2048, DIM=2048, 32 Q heads / 8 KV heads,
HD=64, RoPE, causal) on 8 Trainium2 NeuronCores.

Sharding: hybrid batch x tensor parallel. Core c handles batch b=c//4 and
head-group cp=c%4 (2 KV heads, 8 Q heads). Wq/Wk/Wv are column-sharded.

Output row sharding is interleaved: core d outputs rows [512*R + 64*d,
512*R + 64*d + 64) of BOTH batches for every row chunk R.  That makes the
context AllToAll per-chunk: after chunk R's attention, each core sends its
[feature, 64-row] slices to all 8 dests, so the out-projection for chunk R
runs interleaved into chunk R/R+1's attention stream instead of as a serial
tail after all attention.  Each chunk's A2A is split in two (feature tiles
{0,2} fire mid-chunk after pair 1; {1,3} at chunk end), and the out-proj is
correspondingly split: even-kt chains accumulate in chunk R's second half
(partial sums evicted to SBUF), odd-kt chains + the partial add run in
chunk R+1's first half.  Only chunk 3's odd half remains on the tail.

Wq columns are permuted per core (head blocks [0,4,1,5,2,6,3,7]) so each
score-matmul pair (c, c+4) reads kv heads (0, 1) from the natural kT layout
— no partition-swapped kT_B copy is needed and the two 64-contraction score
matmuls of a pair co-run in disjoint PE row groups.

All matmuls use bf16 inputs with fp32 PSUM accumulation. Activations stay
transposed [feature, token]:
  qT = Wq^T x^T (RoPE on partition dim), kT likewise,
  scoresT[kv, row] = kT^T qT, two kv tiles paired per 2-bank PSUM tile so
  one Exp activation covers 1024 columns,
  probsT = exp(scale*scoresT) in bf16 (no max subtraction: |scores*scale|
  < ~8 for this input distribution; softmax is shift-invariant),
  v is projected feature-major then PE-transposed to token-major with a
  ones column -> partition 64 of the ctx accumulator is the softmax
  denominator for free; it is broadcast with a rank-1 matmul into the SAME
  cacc tile's unused partitions 64..127 (no extra PSUM ring slot),
  reciprocal'd (fast approx) on 64 lanes, and multiplied in.
Every PSUM tile holds exactly one matmul accumulation group at a time —
sequential groups in one bank are fine, interleaved ones clobber.
"""

import numpy as np
from contextlib import ExitStack

import sys

if "/opt/trn_rl_repo" not in sys.path:
    sys.path.insert(0, "/opt/trn_rl_repo")

import ml_dtypes
import concourse.bass as bass
import concourse.bacc as bacc
import concourse.tile as tile
from concourse import mybir
from concourse.bass_utils import run_bass_kernel_spmd
from concourse.masks import make_identity

F32 = mybir.dt.float32
BF16 = mybir.dt.bfloat16
AF = mybir.ActivationFunctionType
NPBF = ml_dtypes.bfloat16

B, S, DIM = 2, 2048, 2048
QH, KVH, HD = 32, 8, 64
SCALE = HD ** -0.5

NCORES = 8
A2A_GROUP = [list(range(NCORES))]
QHL = 8            # q heads per core
KVHL = 2           # kv heads per core
QCOLS = QHL * HD   # 512
KCOLS = KVHL * HD  # 128
TOKC = 512         # token chunk (matmul N / PSUM bank width in fp32)
NTOK = S // TOKC   # 4
KT = DIM // 128    # 16 contraction tiles for the projections
ROWS = 64          # output rows per (core, chunk, batch)
# head-block permutation of the wq columns (block i holds local head PERM[i])
PERM = [0, 4, 1, 5, 2, 6, 3, 7]


def _build_nc():
    nc = bacc.Bacc(None, num_devices=NCORES)

    xq = nc.declare_dram_parameter("xq", [DIM, S], BF16, isOutput=False)
    xk = nc.declare_dram_parameter("xk", [DIM, S], BF16, isOutput=False)
    xv = nc.declare_dram_parameter("xv", [DIM, S], BF16, isOutput=False)
    wq = nc.declare_dram_parameter("wq", [DIM, QCOLS], BF16, isOutput=False)
    wk = nc.declare_dram_parameter("wk", [DIM, KCOLS], BF16, isOutput=False)
    wv = nc.declare_dram_parameter("wv", [DIM, KCOLS], BF16, isOutput=False)
    wo = nc.declare_dram_parameter("wo", [DIM, DIM], BF16, isOutput=False)
    cosT = nc.declare_dram_parameter("cosT", [128, S], BF16, isOutput=False)
    sinT = nc.declare_dram_parameter("sinT", [128, S], BF16, isOutput=False)
    # mask[p, j, w, r] = 1.0 if 128*j + p <= r else 0.0 (causal mask for the
    # 4 diagonal kv tiles of each 512-token row chunk; duplicated along w so
    # one multiply covers both heads of a packed score tile)
    msk = nc.declare_dram_parameter("msk", [128, 4, 2, TOKC], BF16, isOutput=False)
    # out[R, r, :]: r 0..63 -> batch 0 row 512R+64*core+r; 64..127 -> batch 1
    out_ext = nc.declare_dram_parameter("out", [NTOK, 128, DIM], F32, isOutput=True)
    dbg_q = nc.declare_dram_parameter("dbg_q", [128, 4, TOKC], BF16, isOutput=True)
    dbg_k = nc.declare_dram_parameter("dbg_k", [128, TOKC], BF16, isOutput=True)
    dbg_v = nc.declare_dram_parameter("dbg_v", [128, 4, KVHL, HD + 1], BF16, isOutput=True)

    # AllToAll buffers.  Chunks 0..2 use one full buffer per chunk
    # [dest/src, 128, parity, fi, rows] (ftile f = parity + 2*fi); chunk 3
    # is split in halves so its even-kt out-proj can run mid-chunk and only
    # the odd half lands on the tail.
    a2a_inF = [nc.dram_tensor(f"a2a_inF{R}", [NCORES, 128, 2, 2, ROWS], BF16)
               for R in range(NTOK - 1)]
    a2a_outF = [nc.dram_tensor(f"a2a_outF{R}", [NCORES, 128, 2, 2, ROWS], BF16)
                for R in range(NTOK - 1)]
    a2a_in3 = [nc.dram_tensor(f"a2a_in3{h}", [NCORES, 128, 2, ROWS], BF16)
               for h in range(2)]
    a2a_out3 = [nc.dram_tensor(f"a2a_out3{h}", [NCORES, 128, 2, ROWS], BF16)
                for h in range(2)]

    with tile.TileContext(nc) as tc, ExitStack() as ctx:
        const = ctx.enter_context(tc.tile_pool(name="const", bufs=1))
        wpool = ctx.enter_context(tc.tile_pool(name="wpool", bufs=1))
        qkv = ctx.enter_context(tc.tile_pool(name="qkv", bufs=1))
        qtp = ctx.enter_context(tc.tile_pool(name="qtp", bufs=2))
        xstream = ctx.enter_context(tc.tile_pool(name="xstream", bufs=3))
        probs = ctx.enter_context(tc.tile_pool(name="probs", bufs=3))
        ropet = ctx.enter_context(tc.tile_pool(name="ropet", bufs=2))
        denp = ctx.enter_context(tc.tile_pool(name="denp", bufs=4))
        ctxp = ctx.enter_context(tc.tile_pool(name="ctxp", bufs=2))
        ctxf = ctx.enter_context(tc.tile_pool(name="ctxf", bufs=2))
        opartp = ctx.enter_context(tc.tile_pool(name="opart", bufs=1))
        orow_p = ctx.enter_context(tc.tile_pool(name="orow", bufs=2))
        ps_a = ctx.enter_context(tc.tile_pool(name="ps_a", bufs=2, space="PSUM"))
        ps_s = ctx.enter_context(tc.tile_pool(name="ps_s", bufs=2, space="PSUM"))
        ps_c = ctx.enter_context(tc.tile_pool(name="ps_c", bufs=2, space="PSUM"))

        # ---- constants ----
        ones1 = const.tile([1, 64], BF16, tag="ones1")
        nc.vector.memset(ones1, 1.0)
        # identity duplicated in both partition halves for the v transposes
        ident = const.tile([128, 64], BF16, tag="ident")
        make_identity(nc, ident[0:64, :])
        make_identity(nc, ident[64:128, :])

        wq_sb = wpool.tile([128, KT, QCOLS], BF16, tag="wq")
        wk_sb = wpool.tile([128, KT, KCOLS], BF16, tag="wk")
        wv_sb = wpool.tile([128, KT, KCOLS], BF16, tag="wv")
        wo_sb = wpool.tile([128, KT, DIM], BF16, tag="wo")
        cos_sb = const.tile([128, S], BF16, tag="cos")
        sin_sb = const.tile([128, S], BF16, tag="sin")
        msk_sb = const.tile([128, 4, 2, TOKC], BF16, tag="msk")

        # ---- persistent activations ----
        kT_A = qkv.tile([128, S], BF16, tag="ktA", name="ktA")
        # v token-major with a ones column: [tok, kv_tile_idx, kv_head, 65]
        v_sb = qkv.tile([128, S // 128, KVHL, HD + 1], BF16, tag="v")
        nc.vector.memset(v_sb[:, :, :, HD:HD + 1], 1.0)

        xq_r = xq.rearrange("(k2 dt p) c -> p k2 dt c", dt=2, p=128)
        xk_r = xk.rearrange("(k2 dt p) c -> p k2 dt c", dt=2, p=128)
        xv_r = xv.rearrange("(k2 dt p) c -> p k2 dt c", dt=2, p=128)

        def emit_x_dmas_startup():
            """Chunk-0 x DMAs with the weight loads interleaved so the first
            Q matmuls can start ~2us in."""
            tsl = slice(0, TOKC)
            xq_t, xk_t, xv_t = [], [], []
            for k2 in range(KT // 2):
                nc.sync.dma_start(out=wq_sb[:, 2 * k2, :],
                                  in_=wq[(2 * k2) * 128:(2 * k2 + 1) * 128, :])
                nc.sync.dma_start(out=wq_sb[:, 2 * k2 + 1, :],
                                  in_=wq[(2 * k2 + 1) * 128:(2 * k2 + 2) * 128, :])
                t = xstream.tile([128, 2, TOKC], BF16, tag="xqs", bufs=9,
                                 name="xq_t")
                nc.sync.dma_start(out=t, in_=xq_r[:, k2, :, tsl])
                xq_t.append(t)
            nc.sync.dma_start(out=cos_sb[:, 0:TOKC], in_=cosT[:, 0:TOKC])
            nc.sync.dma_start(out=sin_sb[:, 0:TOKC], in_=sinT[:, 0:TOKC])
            nc.sync.dma_start(
                out=wk_sb, in_=wk.rearrange("(kt p) c -> p kt c", p=128))
            for k2 in range(KT // 2):
                t = xstream.tile([128, 2, TOKC], BF16, tag="xks", name="xk_t")
                nc.sync.dma_start(out=t, in_=xk_r[:, k2, :, tsl])
                xk_t.append(t)
            nc.sync.dma_start(
                out=wv_sb, in_=wv.rearrange("(kt p) c -> p kt c", p=128))
            for k2 in range(KT // 2):
                t = xstream.tile([128, 2, TOKC], BF16, tag="xvs", name="xv_t")
                nc.sync.dma_start(out=t, in_=xv_r[:, k2, :, tsl])
                xv_t.append(t)
            nc.sync.dma_start(out=cos_sb[:, TOKC:], in_=cosT[:, TOKC:])
            nc.sync.dma_start(out=sin_sb[:, TOKC:], in_=sinT[:, TOKC:])
            nc.sync.dma_start(out=msk_sb, in_=msk[:, :, :, :])
            return xq_t, xk_t, xv_t

        def alloc_x_tiles(R):
            """Allocates chunk R's x tiles and returns (xts, dma_steps):
            dma_steps[k2] emits the three DMAs for contraction group k2 when
            executed — woven into the proj steps so the DMA queues never hold
            more than a few tiles ahead of the latency-critical a2a sends."""
            tsl = slice(R * TOKC, (R + 1) * TOKC)
            xq_t = [xstream.tile([128, 2, TOKC], BF16, tag="xqs", bufs=9,
                                 name="xq_t") for _ in range(KT // 2)]
            xk_t = [xstream.tile([128, 2, TOKC], BF16, tag="xks", name="xk_t")
                    for _ in range(KT // 2)]
            xv_t = [xstream.tile([128, 2, TOKC], BF16, tag="xvs", name="xv_t")
                    for _ in range(KT // 2)]

            # xk/xv (ring of 3, reused WITHIN the chunk) must be emitted
            # immediately: a slot-reusing DMA emitted before the previous
            # occupant's readers silently clobbers it.  xq's ring of 9 has
            # no same-chunk reuse and its cross-chunk predecessors' readers
            # are all emitted a chunk earlier, so its DMAs can be deferred
            # and woven into the proj steps to keep the queues shallow.
            for k2 in range(KT // 2):
                nc.sync.dma_start(out=xk_t[k2], in_=xk_r[:, k2, :, tsl])
                nc.sync.dma_start(out=xv_t[k2], in_=xv_r[:, k2, :, tsl])
            steps = [lambda k2=k2: nc.sync.dma_start(
                out=xq_t[k2], in_=xq_r[:, k2, :, tsl])
                for k2 in range(KT // 2)]
            return (xq_t, xk_t, xv_t), steps

        def rope_evict(ps, dst, cos_sl, sin_sl):
            """ps: [128, TOKC] fp32 PSUM with fresh projection; dst: bf16
            SBUF tile/slice. dst = ps*cos + rotate_half(ps)*sin."""
            raw = ropet.tile([128, TOKC], BF16, tag="rope_raw")
            nc.scalar.activation(raw, ps, AF.Copy)
            rot = ropet.tile([128, TOKC], BF16, tag="rot")
            for h0 in (0, 64):
                nc.vector.tensor_copy(rot[h0:h0 + 32, :], raw[h0 + 32:h0 + 64, :])
                nc.vector.tensor_copy(rot[h0 + 32:h0 + 64, :], raw[h0:h0 + 32, :])
            t1 = ropet.tile([128, TOKC], BF16, tag="ropet1")
            nc.vector.tensor_mul(t1, raw, cos_sl)
            rot2 = ropet.tile([128, TOKC], BF16, tag="ropet2")
            nc.vector.tensor_mul(rot2, rot, sin_sl)
            nc.vector.tensor_add(dst, t1, rot2)

        def proj_steps(R, xts):
            """Builds chunk R's projection work as ~0.5-1us closures (the x
            DMAs must already be issued via emit_x_dmas)."""
            xq_t, xk_t, xv_t = xts
            tsl = slice(R * TOKC, (R + 1) * TOKC)
            cos_sl = cos_sb[:, tsl]
            sin_sl = sin_sb[:, tsl]

            qts = [qtp.tile([128, TOKC], BF16, tag=f"qt{c}", name=f"qt{c}")
                   for c in range(QCOLS // 128)]
            steps = []

            def q_sweep(cs):
                box = {}

                def mms(k2):
                    if k2 == 0:
                        box['psq'] = [
                            ps_a.tile([128, TOKC], F32, tag="acc", name=f"psq{c}")
                            for c in cs]
                    for dt in range(2):
                        kt = 2 * k2 + dt
                        for i, c in enumerate(cs):
                            nc.tensor.matmul(
                                box['psq'][i],
                                wq_sb[:, kt, c * 128:(c + 1) * 128],
                                xq_t[k2][:, dt, :],
                                start=(kt == 0), stop=(kt == KT - 1))
                for k2 in range(KT // 2):
                    steps.append(lambda k2=k2: mms(k2))
                for i, c in enumerate(cs):
                    steps.append(lambda i=i, c=c:
                                 rope_evict(box['psq'][i], qts[c], cos_sl, sin_sl))

            q_sweep((0, 1))

            kbox = {}

            def k_mms(k2):
                if k2 == 0:
                    kbox['psk'] = ps_a.tile([128, TOKC], F32, tag="acc",
                                            name="psk")
                for dt in range(2):
                    kt = 2 * k2 + dt
                    nc.tensor.matmul(kbox['psk'], wk_sb[:, kt, :],
                                     xk_t[k2][:, dt, :],
                                     start=(kt == 0), stop=(kt == KT - 1))
            for k2 in range(0, KT // 2, 2):
                steps.append(lambda k2=k2: (k_mms(k2), k_mms(k2 + 1)))
            steps.append(lambda: rope_evict(kbox['psk'], kT_A[:, tsl],
                                            cos_sl, sin_sl))

            q_sweep((2, 3))

            vbox = {}

            def v_mms(k2):
                if k2 == 0:
                    vbox['psv'] = ps_a.tile([128, TOKC], F32, tag="acc",
                                            name="psv")
                for dt in range(2):
                    kt = 2 * k2 + dt
                    nc.tensor.matmul(vbox['psv'], wv_sb[:, kt, :],
                                     xv_t[k2][:, dt, :],
                                     start=(kt == 0), stop=(kt == KT - 1))
            for k2 in range(0, KT // 2, 2):
                steps.append(lambda k2=k2: (v_mms(k2), v_mms(k2 + 1)))

            def v_stage():
                vbox['vstage'] = ropet.tile([128, TOKC], BF16, tag="vstage",
                                            name="vstage")
                nc.vector.tensor_copy(vbox['vstage'], vbox['psv'])
            steps.append(v_stage)

            def v_trans(tt):
                vstage = vbox['vstage']
                for h in range(KVHL):
                    pst = ps_a.tile([128, HD], BF16, tag="acc", name="pst")
                    nc.tensor.transpose(
                        pst, vstage[64 * h:64 * h + 64, tt * 128:(tt + 1) * 128],
                        ident[64 * h:64 * h + 64, :])
                    nc.vector.tensor_copy(v_sb[:, R * 4 + tt, h, 0:HD], pst)
            for tt in range(TOKC // 128):
                steps.append(lambda tt=tt: v_trans(tt))

            return steps, qts

        def attention_chunk(R, qts):
            """Returns (part1, part2) step lists for chunk R's attention +
            per-chunk A2A, and (evens, odds) out-proj step lists for chunk
            R's own output rows.  part1 = pairs 0,1 + A2A half 0 (ftiles
            {0,2}); part2 = pairs 2,3 + A2A half 1.  evens must splice after
            part1 begins its gather (we place them in part2's extras); odds
            need A2A half 1 done (splice into chunk R+1 part1, or tail)."""
            nkv = 4 * R + 4
            # ctxt[p, parity, fi, r]: feature tile f = parity + 2*fi
            ctxt = ctxp.tile([128, 2, 2, TOKC], BF16, tag="ctxt", name="ctxt")
            # ctx_full[p, parity, kt2, r]: kt = parity + 2*kt2
            ctx_full = ctxf.tile([128, 2, KT // 2, 128], BF16, tag="ctxf",
                                 name="ctx_full")
            opart = opartp.tile([128, 4, TOKC], F32, tag="opart", name="opart")

            def norm_head(w, c, cacc_w):
                # local head = c (w=0) or c+4 (w=1); its global feature tile
                # is f = c//2 + 2*w -> parity c//2, fi w
                half = c % 2
                den = denp.tile([1, TOKC], BF16, tag="den")
                nc.vector.tensor_copy(den, cacc_w[HD:HD + 1, :])
                # broadcast den at partition base 0 in the score ring (its
                # slot wait targets an earlier-emitted exp, so no deadlock;
                # the custom-DVE reciprocal requires a base-0 PSUM read)
                bc = ps_s.tile([64, TOKC], F32, tag="sc", name="bc")
                nc.tensor.matmul(bc, ones1, den, start=True, stop=True)
                rec = denp.tile([64, TOKC], F32, tag="rec", bufs=2)
                nc.vector.reciprocal_approx_fast(rec, bc)
                nc.vector.tensor_mul(
                    ctxt[64 * half:64 * half + 64, c // 2, w, :],
                    cacc_w[0:HD, :], rec)

            def t_step(c, cacc, t):
                j = t - 4 * R
                trim = 128 * j if j >= 0 else 0
                ksl = slice(t * 128, (t + 1) * 128)
                sc = ps_s.tile([128, 2, TOKC], F32, tag="sc")
                nc.tensor.matmul(sc[:, 0, trim:], kT_A[0:64, ksl],
                                 qts[c][0:64, trim:],
                                 start=True, stop=True)
                nc.tensor.matmul(sc[:, 1, trim:], kT_A[64:128, ksl],
                                 qts[c][64:128, trim:],
                                 start=True, stop=True)
                pr = probs.tile([128, 2, TOKC], BF16, tag="pr")
                nc.scalar.activation(pr[:, :, trim:], sc[:, :, trim:],
                                     AF.Exp, scale=SCALE)
                if j >= 0:
                    nc.vector.tensor_mul(pr[:, :, trim:], pr[:, :, trim:],
                                         msk_sb[:, j, :, trim:])
                for w in range(2):
                    nc.tensor.matmul(cacc[w][0:HD + 1, trim:],
                                     v_sb[:, t, w, :],
                                     pr[:, w, trim:],
                                     start=(t == 0), stop=(t == nkv - 1),
                                     skip_group_check=True)

            def send_half(h):
                # send my ftiles {h, h+2} x 64-row slice for each dest core
                if R < NTOK - 1:
                    for d in range(NCORES):
                        nc.sync.dma_start(
                            out=a2a_inF[R][d][:, h, :, :],
                            in_=ctxt[:, h, :, d * ROWS:(d + 1) * ROWS])
                else:
                    for d in range(NCORES):
                        nc.sync.dma_start(
                            out=a2a_in3[h][d],
                            in_=ctxt[:, h, :, d * ROWS:(d + 1) * ROWS])

            def cc_gather_full():
                # chunks 0..2: one collective moving both halves at chunk end
                nc.gpsimd.collective_compute(
                    "AllToAll", mybir.AluOpType.bypass,
                    replica_groups=A2A_GROUP,
                    ins=[a2a_inF[R][:, :, :, :, :]],
                    outs=[a2a_outF[R][:, :, :, :, :]])
                # gather: src s = (batch b, group s4) -> parity p, kt2
                # {2*s4, 2*s4+1}, rows [64b, 64b+64); split per parity to
                # keep each DMA pattern at <=3 dims
                for s in range(NCORES):
                    b, s4 = divmod(s, 4)
                    for h in range(2):
                        nc.sync.dma_start(
                            out=ctx_full[:, h, 2 * s4:2 * s4 + 2,
                                         b * ROWS:(b + 1) * ROWS],
                            in_=a2a_outF[R][s][:, h])

            def cc_gather_half3(h):
                nc.gpsimd.collective_compute(
                    "AllToAll", mybir.AluOpType.bypass,
                    replica_groups=A2A_GROUP,
                    ins=[a2a_in3[h][:, :, :, :]],
                    outs=[a2a_out3[h][:, :, :, :]])
                for s in range(NCORES):
                    b, s4 = divmod(s, 4)
                    nc.sync.dma_start(
                        out=ctx_full[:, h, 2 * s4:2 * s4 + 2,
                                     b * ROWS:(b + 1) * ROWS],
                        in_=a2a_out3[h][s])

            def build_pairs(cs):
                steps = []
                for c in cs:
                    cacc = [ps_c.tile([128, TOKC], F32, tag="cacc",
                                      name=f"cacc{w}") for w in range(2)]
                    for t in range(nkv):
                        steps.append(lambda c=c, cacc=cacc, t=t:
                                     t_step(c, cacc, t))
                    for w in range(2):
                        steps.append(lambda w=w, c=c, cw=cacc[w]:
                                     norm_head(w, c, cw))
                return steps

            part1 = build_pairs((0, 1))
            part1.append(lambda: send_half(0))
            part2 = build_pairs((2, 3))
            if R < NTOK - 1:
                part2.append(lambda: (send_half(1), cc_gather_full()))
            else:
                part1.append(lambda: cc_gather_half3(0))
                part2.append(lambda: (send_half(1), cc_gather_half3(1)))

            # out-projection for chunk R's 128 output rows (64 per batch)
            obox = {}

            def o_mms(oc, kts, first, last):
                if first:
                    obox[oc] = ps_a.tile([128, TOKC], F32, tag="acc",
                                         name="pso")
                kts = list(kts)
                for kt in kts:
                    nc.tensor.matmul(obox[oc], ctx_full[:, kt % 2, kt // 2, :],
                                     wo_sb[:, kt, oc * TOKC:(oc + 1) * TOKC],
                                     start=(first and kt == kts[0]),
                                     stop=(last and kt == kts[-1]),
                                     skip_group_check=True)

            def fin(oc, add_part):
                orow = orow_p.tile([128, TOKC], F32, tag="orow")
                if add_part:
                    nc.vector.tensor_add(orow, obox[oc], opart[:, oc, :])
                else:
                    nc.vector.tensor_copy(orow, obox[oc])
                nc.sync.dma_start(
                    out=out_ext[R, :, oc * TOKC:(oc + 1) * TOKC], in_=orow)

            if R < NTOK - 1:
                # full 16-kt chains, spliced into chunk R+1 after the
                # collective has had ~a quarter chunk of latency slack
                chains = []
                for oc in range(DIM // TOKC):
                    for i in range(4):
                        chains.append(lambda oc=oc, i=i: o_mms(
                            oc, range(4 * i, 4 * i + 4), i == 0, i == 3))
                    chains.append(lambda oc=oc: fin(oc, False))
                return part1, part2, chains, None
            # chunk 3: even-kt halves run mid-chunk after collective half 0,
            # partials evicted to SBUF; odd halves + add land on the tail
            evens, odds = [], []
            for oc in range(DIM // TOKC):
                evens.append(lambda oc=oc: o_mms(oc, (0, 2, 4, 6), True, False))
                evens.append(lambda oc=oc: o_mms(oc, (8, 10, 12, 14), False, True))
                evens.append(lambda oc=oc:
                             nc.vector.tensor_copy(opart[:, oc, :], obox[oc]))
            for oc in range(DIM // TOKC):
                odds.append(lambda oc=oc: o_mms(oc, (1, 3, 5, 7), True, False))
                odds.append(lambda oc=oc: o_mms(oc, (9, 11, 13, 15), False, True))
                odds.append(lambda oc=oc: fin(oc, True))
            return part1, part2, evens, odds

        def splice(asteps, extras, start_frac=0.0):
            """Runs asteps with extras interleaved evenly; extras begin only
            after start_frac of asteps (latency slack for collectives)."""
            na, ne = len(asteps), len(extras)
            skip = int(na * start_frac)
            eff = max(na - skip, 1)
            j = 0
            for i, st in enumerate(asteps):
                st()
                if i < skip:
                    continue
                while j * eff < (i - skip + 1) * ne:
                    extras[j]()
                    j += 1
            while j < ne:
                extras[j]()
                j += 1

        def weave(psteps, dsteps):
            """Puts the deferred x-DMA steps ahead of the proj steps."""
            return list(dsteps) + list(psteps)

        # ---- chunk 0 projection (serial head, DMA-paced) ----
        xts = emit_x_dmas_startup()
        psteps, qts = proj_steps(0, xts)
        for st in psteps:
            st()

        wo_steps = [lambda kt=kt: nc.sync.dma_start(
            out=wo_sb[:, kt, :], in_=wo[kt * 128:(kt + 1) * 128, :])
            for kt in range(KT)]

        prev_out = []   # out-proj steps of chunk R-1 (full chains, or odds)
        for R in range(NTOK):
            part1, part2, chains, odds = attention_chunk(R, qts)
            if R == 1:
                def dmp(qts=qts):
                    for c4 in range(4):
                        nc.sync.dma_start(out=dbg_q[:, c4, :], in_=qts[c4])
                    nc.sync.dma_start(out=dbg_k[:, :], in_=kT_A[:, TOKC:2*TOKC])
                    nc.sync.dma_start(out=dbg_v[:, :, :, :], in_=v_sb[:, 4:8, :, :])
                part1 = [dmp] + part1
            if R + 1 < NTOK:
                xts, dsteps = alloc_x_tiles(R + 1)
                psteps, qts_next = proj_steps(R + 1, xts)
                psteps = weave(psteps, dsteps)
            else:
                psteps, qts_next = [], None
            if R == 0:
                # all wo tiles must be EMITTED before chunk 0's out-proj
                # chains (which run during chunk 1) — a read emitted before
                # its writer sees uninitialized SBUF
                psteps = psteps + wo_steps
            if R < NTOK - 1:
                # proj first (no collective dep), then chunk R-1's out-proj
                # (lands ~mid-chunk, covering the collective latency), then
                # the proj tail
                q = 2 * len(psteps) // 5
                extras = psteps[:q] + prev_out + psteps[q:]
                splice(part1 + part2, extras)
                prev_out = chains
            else:
                # chunk 3: prev chains in part1; its own even-kt halves in
                # part2 (after collective half 0); odd halves on the tail.
                # start_frac gives the collectives latency slack before the
                # first dependent matmul enters the in-order PE queue
                splice(part1, prev_out, start_frac=0.2)
                splice(part2, chains, start_frac=0.25)
                prev_out = odds
            qts = qts_next
        for st in prev_out:
            st()

    nc.finalize()
    return nc


_NC_CACHE = None


def _get_nc():
    global _NC_CACHE
    if _NC_CACHE is None:
        _NC_CACHE = _build_nc()
    return _NC_CACHE


def _rope_tables():
    idx = np.arange(0, HD, 2, dtype=np.float64) / HD
    inv_freq = 1.0 / 10000.0 ** idx  # RoPE factor branch: adj == 1 here
    pos = np.arange(S, dtype=np.float64)
    freqs = np.einsum("i,j->ij", pos, inv_freq)
    emb = np.concatenate([freqs, freqs], axis=-1)  # [S, HD]
    cos = np.cos(emb).astype(np.float32)
    sin = np.sin(emb).astype(np.float32)
    d = np.arange(128) % HD
    cosT = np.ascontiguousarray(cos[:, d].T)  # [128, S]
    sgn = np.where(d < HD // 2, -1.0, 1.0).astype(np.float32)
    sinT = np.ascontiguousarray(sin[:, d].T * sgn[:, None])
    return cosT.astype(NPBF), sinT.astype(NPBF)


def _masks():
    p = np.arange(128)[:, None]
    r = np.arange(TOKC)[None, :]
    m = np.stack([(128 * j + p <= r) for j in range(4)], axis=1)
    m = np.repeat(m[:, :, None, :], 2, axis=2)
    return np.ascontiguousarray(m.astype(NPBF))  # [128, 4, 2, TOKC]


def kernel(query, key, value, w_q, b_q, w_k, b_k, w_v, b_v, w_o, b_o,
           _trace=False, **_unused):
    for b in (b_q, b_k, b_v):
        assert np.abs(np.asarray(b)).max() == 0.0, "nonzero qkv bias unsupported"

    cosT, sinT = _rope_tables()
    msk = _masks()
    xqT = [np.ascontiguousarray(np.asarray(query)[b].T).astype(NPBF) for b in range(B)]
    xkT = [np.ascontiguousarray(np.asarray(key)[b].T).astype(NPBF) for b in range(B)]
    xvT = [np.ascontiguousarray(np.asarray(value)[b].T).astype(NPBF) for b in range(B)]
    w_q, w_k, w_v, w_o = (np.asarray(a) for a in (w_q, w_k, w_v, w_o))
    wo_bf = np.ascontiguousarray(w_o).astype(NPBF)

    in_maps = []
    for c in range(NCORES):
        b, cp = divmod(c, 4)
        wq_c = w_q[:, cp * QCOLS:(cp + 1) * QCOLS]
        wq_c = wq_c.reshape(DIM, QHL, HD)[:, PERM, :].reshape(DIM, QCOLS)
        in_maps.append({
            "xq": xqT[b], "xk": xkT[b], "xv": xvT[b],
            "wq": np.ascontiguousarray(wq_c).astype(NPBF),
            "wk": np.ascontiguousarray(w_k[:, cp * KCOLS:(cp + 1) * KCOLS]).astype(NPBF),
            "wv": np.ascontiguousarray(w_v[:, cp * KCOLS:(cp + 1) * KCOLS]).astype(NPBF),
            "wo": wo_bf,
            "cosT": cosT, "sinT": sinT, "msk": msk,
        })

    nc = _get_nc()
    res = run_bass_kernel_spmd(nc, in_maps, list(range(NCORES)), trace=_trace)
    out = np.empty((B, S, DIM), np.float32)
    for c in range(NCORES):
        r = res.results[c]["out"]  # [NTOK, 128, DIM]
        for R in range(NTOK):
            rows = slice(TOKC * R + ROWS * c, TOKC * R + ROWS * (c + 1))
            out[0, rows, :] = r[R, 0:ROWS, :]
            out[1, rows, :] = r[R, ROWS:2 * ROWS, :]
    out += np.asarray(b_o)[None, None, :]
    if _trace:
        return out, res
    return out


# revision 38
# speedup vs baseline: 1.0859x; 1.0859x over previous
"""Grouped-Query Attention (B=2, S=2048, DIM=2048, 32 Q heads / 8 KV heads,
HD=64, RoPE, causal) on 8 Trainium2 NeuronCores.

Sharding: hybrid batch x tensor parallel. Core c handles batch b=c//4 and
head-group cp=c%4 (2 KV heads, 8 Q heads). Wq/Wk/Wv are column-sharded.

Output row sharding is interleaved: core d outputs rows [512*R + 64*d,
512*R + 64*d + 64) of BOTH batches for every row chunk R.  That makes the
context AllToAll per-chunk: after chunk R's attention, each core sends its
[feature, 64-row] slices to all 8 dests, so the out-projection for chunk R
runs interleaved into chunk R/R+1's attention stream instead of as a serial
tail after all attention.  Each chunk's A2A is split in two (feature tiles
{0,2} fire mid-chunk after pair 1; {1,3} at chunk end), and the out-proj is
correspondingly split: even-kt chains accumulate in chunk R's second half
(partial sums evicted to SBUF), odd-kt chains + the partial add run in
chunk R+1's first half.  Only chunk 3's odd half remains on the tail.

Wq columns are permuted per core (head blocks [0,4,1,5,2,6,3,7]) so each
score-matmul pair (c, c+4) reads kv heads (0, 1) from the natural kT layout
— no partition-swapped kT_B copy is needed and the two 64-contraction score
matmuls of a pair co-run in disjoint PE row groups.

All matmuls use bf16 inputs with fp32 PSUM accumulation. Activations stay
transposed [feature, token]:
  qT = Wq^T x^T (RoPE on partition dim), kT likewise,
  scoresT[kv, row] = kT^T qT, two kv tiles paired per 2-bank PSUM tile so
  one Exp activation covers 1024 columns,
  probsT = exp(scale*scoresT) in bf16 (no max subtraction: |scores*scale|
  < ~8 for this input distribution; softmax is shift-invariant),
  v is projected feature-major then PE-transposed to token-major with a
  ones column -> partition 64 of the ctx accumulator is the softmax
  denominator for free; it is broadcast with a rank-1 matmul into the SAME
  cacc tile's unused partitions 64..127 (no extra PSUM ring slot),
  reciprocal'd (fast approx) on 64 lanes, and multiplied in.
Every PSUM tile holds exactly one matmul accumulation group at a time —
sequential groups in one bank are fine, interleaved ones clobber.
"""

import numpy as np
from contextlib import ExitStack

import sys

if "/opt/trn_rl_repo" not in sys.path:
    sys.path.insert(0, "/opt/trn_rl_repo")

import ml_dtypes
import concourse.bass as bass
import concourse.bacc as bacc
import concourse.tile as tile
from concourse import mybir
from concourse.bass_utils import run_bass_kernel_spmd
from concourse.masks import make_identity

F32 = mybir.dt.float32
BF16 = mybir.dt.bfloat16
AF = mybir.ActivationFunctionType
NPBF = ml_dtypes.bfloat16

B, S, DIM = 2, 2048, 2048
QH, KVH, HD = 32, 8, 64
SCALE = HD ** -0.5

NCORES = 8
A2A_GROUP = [list(range(NCORES))]
QHL = 8            # q heads per core
KVHL = 2           # kv heads per core
QCOLS = QHL * HD   # 512
KCOLS = KVHL * HD  # 128
TOKC = 512         # token chunk (matmul N / PSUM bank width in fp32)
NTOK = S // TOKC   # 4
KT = DIM // 128    # 16 contraction tiles for the projections
ROWS = 64          # output rows per (core, chunk, batch)
# head-block permutation of the wq columns (block i holds local head PERM[i])
PERM = [0, 4, 1, 5, 2, 6, 3, 7]


def _build_nc():
    nc = bacc.Bacc(None, num_devices=NCORES)

    xq = nc.declare_dram_parameter("xq", [DIM, S], BF16, isOutput=False)
    xk = nc.declare_dram_parameter("xk", [DIM, S], BF16, isOutput=False)
    xv = nc.declare_dram_parameter("xv", [DIM, S], BF16, isOutput=False)
    wq = nc.declare_dram_parameter("wq", [DIM, QCOLS], BF16, isOutput=False)
    wk = nc.declare_dram_parameter("wk", [DIM, KCOLS], BF16, isOutput=False)
    wv = nc.declare_dram_parameter("wv", [DIM, KCOLS], BF16, isOutput=False)
    wo = nc.declare_dram_parameter("wo", [DIM, DIM], BF16, isOutput=False)
    cosT = nc.declare_dram_parameter("cosT", [128, S], BF16, isOutput=False)
    sinT = nc.declare_dram_parameter("sinT", [128, S], BF16, isOutput=False)
    # mask[p, j, w, r] = 1.0 if 128*j + p <= r else 0.0 (causal mask for the
    # 4 diagonal kv tiles of each 512-token row chunk; duplicated along w so
    # one multiply covers both heads of a packed score tile)
    msk = nc.declare_dram_parameter("msk", [128, 4, 2, TOKC], BF16, isOutput=False)
    # out[R, r, :]: r 0..63 -> batch 0 row 512R+64*core+r; 64..127 -> batch 1
    out_ext = nc.declare_dram_parameter("out", [NTOK, 128, DIM], F32, isOutput=True)

    # AllToAll buffers.  Chunks 0..2 use one full buffer per chunk
    # [dest/src, 128, parity, fi, rows] (ftile f = parity + 2*fi); chunk 3
    # is split in halves so its even-kt out-proj can run mid-chunk and only
    # the odd half lands on the tail.
    a2a_inF = [nc.dram_tensor(f"a2a_inF{R}", [NCORES, 128, 2, 2, ROWS], BF16)
               for R in range(NTOK - 1)]
    a2a_outF = [nc.dram_tensor(f"a2a_outF{R}", [NCORES, 128, 2, 2, ROWS], BF16)
                for R in range(NTOK - 1)]
    a2a_in3 = [nc.dram_tensor(f"a2a_in3{h}", [NCORES, 128, 2, ROWS], BF16)
               for h in range(2)]
    a2a_out3 = [nc.dram_tensor(f"a2a_out3{h}", [NCORES, 128, 2, ROWS], BF16)
                for h in range(2)]

    with tile.TileContext(nc) as tc, ExitStack() as ctx:
        const = ctx.enter_context(tc.tile_pool(name="const", bufs=1))
        wpool = ctx.enter_context(tc.tile_pool(name="wpool", bufs=1))
        qkv = ctx.enter_context(tc.tile_pool(name="qkv", bufs=1))
        qtp = ctx.enter_context(tc.tile_pool(name="qtp", bufs=2))
        xstream = ctx.enter_context(tc.tile_pool(name="xstream", bufs=3))
        probs = ctx.enter_context(tc.tile_pool(name="probs", bufs=3))
        ropet = ctx.enter_context(tc.tile_pool(name="ropet", bufs=2))
        denp = ctx.enter_context(tc.tile_pool(name="denp", bufs=4))
        ctxp = ctx.enter_context(tc.tile_pool(name="ctxp", bufs=2))
        ctxf = ctx.enter_context(tc.tile_pool(name="ctxf", bufs=2))
        opartp = ctx.enter_context(tc.tile_pool(name="opart", bufs=1))
        orow_p = ctx.enter_context(tc.tile_pool(name="orow", bufs=2))
        ps_a = ctx.enter_context(tc.tile_pool(name="ps_a", bufs=2, space="PSUM"))
        ps_s = ctx.enter_context(tc.tile_pool(name="ps_s", bufs=2, space="PSUM"))
        ps_c = ctx.enter_context(tc.tile_pool(name="ps_c", bufs=2, space="PSUM"))

        # ---- constants ----
        ones1 = const.tile([1, 64], BF16, tag="ones1")
        nc.vector.memset(ones1, 1.0)
        # identity duplicated in both partition halves for the v transposes
        ident = const.tile([128, 64], BF16, tag="ident")
        make_identity(nc, ident[0:64, :])
        make_identity(nc, ident[64:128, :])

        wq_sb = wpool.tile([128, KT, QCOLS], BF16, tag="wq")
        wk_sb = wpool.tile([128, KT, KCOLS], BF16, tag="wk")
        wv_sb = wpool.tile([128, KT, KCOLS], BF16, tag="wv")
        wo_sb = wpool.tile([128, KT, DIM], BF16, tag="wo")
        cos_sb = const.tile([128, S], BF16, tag="cos")
        sin_sb = const.tile([128, S], BF16, tag="sin")
        msk_sb = const.tile([128, 4, 2, TOKC], BF16, tag="msk")

        # ---- persistent activations ----
        kT_A = qkv.tile([128, S], BF16, tag="ktA", name="ktA")
        # v token-major with a ones column: [tok, kv_tile_idx, kv_head, 65]
        v_sb = qkv.tile([128, S // 128, KVHL, HD + 1], BF16, tag="v")
        nc.vector.memset(v_sb[:, :, :, HD:HD + 1], 1.0)

        xq_r = xq.rearrange("(k2 dt p) c -> p k2 dt c", dt=2, p=128)
        xk_r = xk.rearrange("(k2 dt p) c -> p k2 dt c", dt=2, p=128)
        xv_r = xv.rearrange("(k2 dt p) c -> p k2 dt c", dt=2, p=128)

        def emit_x_dmas_startup():
            """Chunk-0 x DMAs with the weight loads interleaved so the first
            Q matmuls can start ~2us in."""
            tsl = slice(0, TOKC)
            xq_t, xk_t, xv_t = [], [], []
            for k2 in range(KT // 2):
                nc.sync.dma_start(out=wq_sb[:, 2 * k2, :],
                                  in_=wq[(2 * k2) * 128:(2 * k2 + 1) * 128, :])
                nc.sync.dma_start(out=wq_sb[:, 2 * k2 + 1, :],
                                  in_=wq[(2 * k2 + 1) * 128:(2 * k2 + 2) * 128, :])
                t = xstream.tile([128, 2, TOKC], BF16, tag="xqs", bufs=9,
                                 name="xq_t")
                nc.sync.dma_start(out=t, in_=xq_r[:, k2, :, tsl])
                xq_t.append(t)
            nc.sync.dma_start(out=cos_sb[:, 0:TOKC], in_=cosT[:, 0:TOKC])
            nc.sync.dma_start(out=sin_sb[:, 0:TOKC], in_=sinT[:, 0:TOKC])
            nc.sync.dma_start(
                out=wk_sb, in_=wk.rearrange("(kt p) c -> p kt c", p=128))
            for k2 in range(KT // 2):
                t = xstream.tile([128, 2, TOKC], BF16, tag="xks", name="xk_t")
                nc.sync.dma_start(out=t, in_=xk_r[:, k2, :, tsl])
                xk_t.append(t)
            nc.sync.dma_start(
                out=wv_sb, in_=wv.rearrange("(kt p) c -> p kt c", p=128))
            for k2 in range(KT // 2):
                t = xstream.tile([128, 2, TOKC], BF16, tag="xvs", name="xv_t")
                nc.sync.dma_start(out=t, in_=xv_r[:, k2, :, tsl])
                xv_t.append(t)
            nc.sync.dma_start(out=cos_sb[:, TOKC:], in_=cosT[:, TOKC:])
            nc.sync.dma_start(out=sin_sb[:, TOKC:], in_=sinT[:, TOKC:])
            nc.sync.dma_start(out=msk_sb, in_=msk[:, :, :, :])
            return xq_t, xk_t, xv_t

        def alloc_x_tiles(R):
            """Allocates chunk R's x tiles and returns (xts, dma_steps):
            dma_steps[k2] emits the three DMAs for contraction group k2 when
            executed — woven into the proj steps so the DMA queues never hold
            more than a few tiles ahead of the latency-critical a2a sends."""
            tsl = slice(R * TOKC, (R + 1) * TOKC)
            xq_t = [xstream.tile([128, 2, TOKC], BF16, tag="xqs", bufs=9,
                                 name="xq_t") for _ in range(KT // 2)]
            xk_t = [xstream.tile([128, 2, TOKC], BF16, tag="xks", name="xk_t")
                    for _ in range(KT // 2)]
            xv_t = [xstream.tile([128, 2, TOKC], BF16, tag="xvs", name="xv_t")
                    for _ in range(KT // 2)]

            # xk/xv (ring of 3, reused WITHIN the chunk) must be emitted
            # immediately: a slot-reusing DMA emitted before the previous
            # occupant's readers silently clobbers it.  xq's ring of 9 has
            # no same-chunk reuse and its cross-chunk predecessors' readers
            # are all emitted a chunk earlier, so its DMAs can be deferred
            # and woven into the proj steps to keep the queues shallow.
            for k2 in range(KT // 2):
                nc.sync.dma_start(out=xk_t[k2], in_=xk_r[:, k2, :, tsl])
                nc.sync.dma_start(out=xv_t[k2], in_=xv_r[:, k2, :, tsl])
            steps = [lambda k2=k2: nc.sync.dma_start(
                out=xq_t[k2], in_=xq_r[:, k2, :, tsl])
                for k2 in range(KT // 2)]
            return (xq_t, xk_t, xv_t), steps

        def rope_evict(ps, dst, cos_sl, sin_sl):
            """ps: [128, TOKC] fp32 PSUM with fresh projection; dst: bf16
            SBUF tile/slice. dst = ps*cos + rotate_half(ps)*sin."""
            raw = ropet.tile([128, TOKC], BF16, tag="rope_raw")
            nc.scalar.activation(raw, ps, AF.Copy)
            rot = ropet.tile([128, TOKC], BF16, tag="rot")
            for h0 in (0, 64):
                nc.vector.tensor_copy(rot[h0:h0 + 32, :], raw[h0 + 32:h0 + 64, :])
                nc.vector.tensor_copy(rot[h0 + 32:h0 + 64, :], raw[h0:h0 + 32, :])
            t1 = ropet.tile([128, TOKC], BF16, tag="ropet1")
            nc.vector.tensor_mul(t1, raw, cos_sl)
            rot2 = ropet.tile([128, TOKC], BF16, tag="ropet2")
            nc.vector.tensor_mul(rot2, rot, sin_sl)
            nc.vector.tensor_add(dst, t1, rot2)

        def proj_steps(R, xts):
            """Builds chunk R's projection work as ~0.5-1us closures (the x
            DMAs must already be issued via emit_x_dmas)."""
            xq_t, xk_t, xv_t = xts
            tsl = slice(R * TOKC, (R + 1) * TOKC)
            cos_sl = cos_sb[:, tsl]
            sin_sl = sin_sb[:, tsl]

            qts = [qtp.tile([128, TOKC], BF16, tag=f"qt{c}", name=f"qt{c}")
                   for c in range(QCOLS // 128)]
            steps = []

            def q_sweep(cs):
                box = {}

                def mms(k2):
                    if k2 == 0:
                        box['psq'] = [
                            ps_a.tile([128, TOKC], F32, tag="acc", name=f"psq{c}")
                            for c in cs]
                    for dt in range(2):
                        kt = 2 * k2 + dt
                        for i, c in enumerate(cs):
                            nc.tensor.matmul(
                                box['psq'][i],
                                wq_sb[:, kt, c * 128:(c + 1) * 128],
                                xq_t[k2][:, dt, :],
                                start=(kt == 0), stop=(kt == KT - 1))
                for k2 in range(KT // 2):
                    steps.append(lambda k2=k2: mms(k2))
                for i, c in enumerate(cs):
                    steps.append(lambda i=i, c=c:
                                 rope_evict(box['psq'][i], qts[c], cos_sl, sin_sl))

            q_sweep((0, 1))

            kbox = {}

            def k_mms(k2):
                if k2 == 0:
                    kbox['psk'] = ps_a.tile([128, TOKC], F32, tag="acc",
                                            name="psk")
                for dt in range(2):
                    kt = 2 * k2 + dt
                    nc.tensor.matmul(kbox['psk'], wk_sb[:, kt, :],
                                     xk_t[k2][:, dt, :],
                                     start=(kt == 0), stop=(kt == KT - 1))
            for k2 in range(0, KT // 2, 2):
                steps.append(lambda k2=k2: (k_mms(k2), k_mms(k2 + 1)))
            steps.append(lambda: rope_evict(kbox['psk'], kT_A[:, tsl],
                                            cos_sl, sin_sl))

            q_sweep((2, 3))

            vbox = {}

            def v_mms(k2):
                if k2 == 0:
                    vbox['psv'] = ps_a.tile([128, TOKC], F32, tag="acc",
                                            name="psv")
                for dt in range(2):
                    kt = 2 * k2 + dt
                    nc.tensor.matmul(vbox['psv'], wv_sb[:, kt, :],
                                     xv_t[k2][:, dt, :],
                                     start=(kt == 0), stop=(kt == KT - 1))
            for k2 in range(0, KT // 2, 2):
                steps.append(lambda k2=k2: (v_mms(k2), v_mms(k2 + 1)))

            def v_stage():
                vbox['vstage'] = ropet.tile([128, TOKC], BF16, tag="vstage",
                                            name="vstage")
                nc.vector.tensor_copy(vbox['vstage'], vbox['psv'])
            steps.append(v_stage)

            def v_trans(tt):
                vstage = vbox['vstage']
                for h in range(KVHL):
                    pst = ps_a.tile([128, HD], BF16, tag="acc", name="pst")
                    nc.tensor.transpose(
                        pst, vstage[64 * h:64 * h + 64, tt * 128:(tt + 1) * 128],
                        ident[64 * h:64 * h + 64, :])
                    nc.vector.tensor_copy(v_sb[:, R * 4 + tt, h, 0:HD], pst)
            for tt in range(TOKC // 128):
                steps.append(lambda tt=tt: v_trans(tt))

            return steps, qts

        def attention_chunk(R, qts):
            """Returns (part1, part2) step lists for chunk R's attention +
            per-chunk A2A, and (evens, odds) out-proj step lists for chunk
            R's own output rows.  part1 = pairs 0,1 + A2A half 0 (ftiles
            {0,2}); part2 = pairs 2,3 + A2A half 1.  evens must splice after
            part1 begins its gather (we place them in part2's extras); odds
            need A2A half 1 done (splice into chunk R+1 part1, or tail)."""
            nkv = 4 * R + 4
            # ctxt[p, parity, fi, r]: feature tile f = parity + 2*fi
            ctxt = ctxp.tile([128, 2, 2, TOKC], BF16, tag="ctxt", name="ctxt")
            # ctx_full[p, parity, kt2, r]: kt = parity + 2*kt2
            ctx_full = ctxf.tile([128, 2, KT // 2, 128], BF16, tag="ctxf",
                                 name="ctx_full")
            opart = opartp.tile([128, 4, TOKC], F32, tag="opart", name="opart")

            def norm_head(w, c, cacc_w):
                # local head = c (w=0) or c+4 (w=1); its global feature tile
                # is f = c//2 + 2*w -> parity c//2, fi w
                half = c % 2
                den = denp.tile([1, TOKC], BF16, tag="den")
                nc.vector.tensor_copy(den, cacc_w[HD:HD + 1, :])
                # broadcast den at partition base 0 in the score ring (its
                # slot wait targets an earlier-emitted exp, so no deadlock;
                # the custom-DVE reciprocal requires a base-0 PSUM read)
                bc = ps_s.tile([64, TOKC], F32, tag="sc", name="bc")
                nc.tensor.matmul(bc, ones1, den, start=True, stop=True)
                rec = denp.tile([64, TOKC], F32, tag="rec", bufs=2)
                nc.vector.reciprocal_approx_fast(rec, bc)
                nc.vector.tensor_mul(
                    ctxt[64 * half:64 * half + 64, c // 2, w, :],
                    cacc_w[0:HD, :], rec)

            def t_step(c, cacc, t):
                j = t - 4 * R
                trim = 128 * j if j >= 0 else 0
                ksl = slice(t * 128, (t + 1) * 128)
                sc = ps_s.tile([128, 2, TOKC], F32, tag="sc")
                nc.tensor.matmul(sc[:, 0, trim:], kT_A[0:64, ksl],
                                 qts[c][0:64, trim:],
                                 start=True, stop=True)
                nc.tensor.matmul(sc[:, 1, trim:], kT_A[64:128, ksl],
                                 qts[c][64:128, trim:],
                                 start=True, stop=True)
                pr = probs.tile([128, 2, TOKC], BF16, tag="pr")
                nc.scalar.activation(pr[:, :, trim:], sc[:, :, trim:],
                                     AF.Exp, scale=SCALE)
                if j >= 0:
                    nc.vector.tensor_mul(pr[:, :, trim:], pr[:, :, trim:],
                                         msk_sb[:, j, :, trim:])
                for w in range(2):
                    nc.tensor.matmul(cacc[w][0:HD + 1, trim:],
                                     v_sb[:, t, w, :],
                                     pr[:, w, trim:],
                                     start=(t == 0), stop=(t == nkv - 1),
                                     skip_group_check=True)

            def send_half(h):
                # send my ftiles {h, h+2} x 64-row slice for each dest core
                if R < NTOK - 1:
                    for d in range(NCORES):
                        nc.sync.dma_start(
                            out=a2a_inF[R][d][:, h, :, :],
                            in_=ctxt[:, h, :, d * ROWS:(d + 1) * ROWS])
                else:
                    for d in range(NCORES):
                        nc.sync.dma_start(
                            out=a2a_in3[h][d],
                            in_=ctxt[:, h, :, d * ROWS:(d + 1) * ROWS])

            def cc_gather_full():
                # chunks 0..2: one collective moving both halves at chunk end
                nc.gpsimd.collective_compute(
                    "AllToAll", mybir.AluOpType.bypass,
                    replica_groups=A2A_GROUP,
                    ins=[a2a_inF[R][:, :, :, :, :]],
                    outs=[a2a_outF[R][:, :, :, :, :]])
                # gather: src s = (batch b, group s4) -> parity p, kt2
                # {2*s4, 2*s4+1}, rows [64b, 64b+64); split per parity to
                # keep each DMA pattern at <=3 dims
                for s in range(NCORES):
                    b, s4 = divmod(s, 4)
                    for h in range(2):
                        nc.sync.dma_start(
                            out=ctx_full[:, h, 2 * s4:2 * s4 + 2,
                                         b * ROWS:(b + 1) * ROWS],
                            in_=a2a_outF[R][s][:, h])

            def cc_gather_half3(h):
                nc.gpsimd.collective_compute(
                    "AllToAll", mybir.AluOpType.bypass,
                    replica_groups=A2A_GROUP,
                    ins=[a2a_in3[h][:, :, :, :]],
                    outs=[a2a_out3[h][:, :, :, :]])
                for s in range(NCORES):
                    b, s4 = divmod(s, 4)
                    nc.sync.dma_start(
                        out=ctx_full[:, h, 2 * s4:2 * s4 + 2,
                                     b * ROWS:(b + 1) * ROWS],
                        in_=a2a_out3[h][s])

            def build_pairs(cs):
                steps = []
                for c in cs:
                    cacc = [ps_c.tile([128, TOKC], F32, tag="cacc",
                                      name=f"cacc{w}") for w in range(2)]
                    for t in range(nkv):
                        steps.append(lambda c=c, cacc=cacc, t=t:
                                     t_step(c, cacc, t))
                    for w in range(2):
                        steps.append(lambda w=w, c=c, cw=cacc[w]:
                                     norm_head(w, c, cw))
                return steps

            part1 = build_pairs((0, 1))
            part1.append(lambda: send_half(0))
            part2 = build_pairs((2, 3))
            if R < NTOK - 1:
                part2.append(lambda: (send_half(1), cc_gather_full()))
            else:
                part1.append(lambda: cc_gather_half3(0))
                part2.append(lambda: (send_half(1), cc_gather_half3(1)))

            # out-projection for chunk R's 128 output rows (64 per batch)
            obox = {}

            def o_mms(oc, kts, first, last):
                if first:
                    obox[oc] = ps_a.tile([128, TOKC], F32, tag="acc",
                                         name="pso")
                kts = list(kts)
                for kt in kts:
                    nc.tensor.matmul(obox[oc], ctx_full[:, kt % 2, kt // 2, :],
                                     wo_sb[:, kt, oc * TOKC:(oc + 1) * TOKC],
                                     start=(first and kt == kts[0]),
                                     stop=(last and kt == kts[-1]),
                                     skip_group_check=True)

            def fin(oc, add_part):
                orow = orow_p.tile([128, TOKC], F32, tag="orow")
                if add_part:
                    nc.vector.tensor_add(orow, obox[oc], opart[:, oc, :])
                else:
                    nc.vector.tensor_copy(orow, obox[oc])
                nc.sync.dma_start(
                    out=out_ext[R, :, oc * TOKC:(oc + 1) * TOKC], in_=orow)

            if R < NTOK - 1:
                # full 16-kt chains, spliced into chunk R+1 after the
                # collective has had ~a quarter chunk of latency slack
                chains = []
                for oc in range(DIM // TOKC):
                    for i in range(4):
                        chains.append(lambda oc=oc, i=i: o_mms(
                            oc, range(4 * i, 4 * i + 4), i == 0, i == 3))
                    chains.append(lambda oc=oc: fin(oc, False))
                return part1, part2, chains, None
            # chunk 3: even-kt halves run mid-chunk after collective half 0,
            # partials evicted to SBUF; odd halves + add land on the tail
            evens, odds = [], []
            for oc in range(DIM // TOKC):
                evens.append(lambda oc=oc: o_mms(oc, (0, 2, 4, 6), True, False))
                evens.append(lambda oc=oc: o_mms(oc, (8, 10, 12, 14), False, True))
                evens.append(lambda oc=oc:
                             nc.vector.tensor_copy(opart[:, oc, :], obox[oc]))
            for oc in range(DIM // TOKC):
                odds.append(lambda oc=oc: o_mms(oc, (1, 3, 5, 7), True, False))
                odds.append(lambda oc=oc: o_mms(oc, (9, 11, 13, 15), False, True))
                odds.append(lambda oc=oc: fin(oc, True))
            return part1, part2, evens, odds

        def splice(asteps, extras, start_frac=0.0):
            """Runs asteps with extras interleaved evenly; extras begin only
            after start_frac of asteps (latency slack for collectives)."""
            na, ne = len(asteps), len(extras)
            skip = int(na * start_frac)
            eff = max(na - skip, 1)
            j = 0
            for i, st in enumerate(asteps):
                st()
                if i < skip:
                    continue
                while j * eff < (i - skip + 1) * ne:
                    extras[j]()
                    j += 1
            while j < ne:
                extras[j]()
                j += 1

        def weave(psteps, dsteps):
            """Puts the deferred x-DMA steps ahead of the proj steps."""
            return list(dsteps) + list(psteps)

        # ---- chunk 0 projection (serial head, DMA-paced) ----
        xts = emit_x_dmas_startup()
        psteps, qts = proj_steps(0, xts)
        for st in psteps:
            st()

        wo_steps = [lambda kt=kt: nc.sync.dma_start(
            out=wo_sb[:, kt, :], in_=wo[kt * 128:(kt + 1) * 128, :])
            for kt in range(KT)]

        prev_out = []   # out-proj steps of chunk R-1 (full chains, or odds)
        for R in range(NTOK):
            part1, part2, chains, odds = attention_chunk(R, qts)
            if R + 1 < NTOK:
                xts, dsteps = alloc_x_tiles(R + 1)
                psteps, qts_next = proj_steps(R + 1, xts)
                psteps = weave(psteps, dsteps)
            else:
                psteps, qts_next = [], None
            if R == 0:
                # all wo tiles must be EMITTED before chunk 0's out-proj
                # chains (which run during chunk 1) — a read emitted before
                # its writer sees uninitialized SBUF
                psteps = psteps + wo_steps
            if R < NTOK - 1:
                # proj first (no collective dep), then chunk R-1's out-proj
                # (lands ~mid-chunk, covering the collective latency), then
                # the proj tail
                q = 2 * len(psteps) // 5
                extras = psteps[:q] + prev_out + psteps[q:]
                splice(part1 + part2, extras)
                prev_out = chains
            else:
                # chunk 3: prev chains in part1; its own even-kt halves in
                # part2 (after collective half 0); odd halves on the tail.
                # start_frac gives the collectives latency slack before the
                # first dependent matmul enters the in-order PE queue
                splice(part1, prev_out, start_frac=0.2)
                splice(part2, chains, start_frac=0.25)
                prev_out = odds
            qts = qts_next
        for st in prev_out:
            st()

    nc.finalize()
    return nc


_NC_CACHE = None


def _get_nc():
    global _NC_CACHE
    if _NC_CACHE is None:
        _NC_CACHE = _build_nc()
    return _NC_CACHE


def _rope_tables():
    idx = np.arange(0, HD, 2, dtype=np.float64) / HD
    inv_freq = 1.0 / 10000.0 ** idx  # RoPE factor branch: adj == 1 here
    pos = np.arange(S, dtype=np.float64)
    freqs = np.einsum("i,j->ij", pos, inv_freq)
    emb = np.concatenate([freqs, freqs], axis=-1)  # [S, HD]
    cos = np.cos(emb).astype(np.float32)
    sin = np.sin(emb).astype(np.float32)
    d = np.arange(128) % HD
    cosT = np.ascontiguousarray(cos[:, d].T)  # [128, S]
    sgn = np.where(d < HD // 2, -1.0, 1.0).astype(np.float32)
    sinT = np.ascontiguousarray(sin[:, d].T * sgn[:, None])
    return cosT.astype(NPBF), sinT.astype(NPBF)


def _masks():
    p = np.arange(128)[:, None]
    r = np.arange(TOKC)[None, :]
    m = np.stack([(128 * j + p <= r) for j in range(4)], axis=1)
    m = np.repeat(m[:, :, None, :], 2, axis=2)
    return np.ascontiguousarray(m.astype(NPBF))  # [128, 4, 2, TOKC]


def kernel(query, key, value, w_q, b_q, w_k, b_k, w_v, b_v, w_o, b_o,
           _trace=False, **_unused):
    for b in (b_q, b_k, b_v):
        assert np.abs(np.asarray(b)).max() == 0.0, "nonzero qkv bias unsupported"

    cosT, sinT = _rope_tables()
    msk = _masks()
    xqT = [np.ascontiguousarray(np.asarray(query)[b].T).astype(NPBF) for b in range(B)]
    xkT = [np.ascontiguousarray(np.asarray(key)[b].T).astype(NPBF) for b in range(B)]
    xvT = [np.ascontiguousarray(np.asarray(value)[b].T).astype(NPBF) for b in range(B)]
    w_q, w_k, w_v, w_o = (np.asarray(a) for a in (w_q, w_k, w_v, w_o))
    wo_bf = np.ascontiguousarray(w_o).astype(NPBF)

    in_maps = []
    for c in range(NCORES):
        b, cp = divmod(c, 4)
        wq_c = w_q[:, cp * QCOLS:(cp + 1) * QCOLS]
        wq_c = wq_c.reshape(DIM, QHL, HD)[:, PERM, :].reshape(DIM, QCOLS)
        in_maps.append({
            "xq": xqT[b], "xk": xkT[b], "xv": xvT[b],
            "wq": np.ascontiguousarray(wq_c).astype(NPBF),
            "wk": np.ascontiguousarray(w_k[:, cp * KCOLS:(cp + 1) * KCOLS]).astype(NPBF),
            "wv": np.ascontiguousarray(w_v[:, cp * KCOLS:(cp + 1) * KCOLS]).astype(NPBF),
            "wo": wo_bf,
            "cosT": cosT, "sinT": sinT, "msk": msk,
        })

    nc = _get_nc()
    res = run_bass_kernel_spmd(nc, in_maps, list(range(NCORES)), trace=_trace)
    out = np.empty((B, S, DIM), np.float32)
    for c in range(NCORES):
        r = res.results[c]["out"]  # [NTOK, 128, DIM]
        for R in range(NTOK):
            rows = slice(TOKC * R + ROWS * c, TOKC * R + ROWS * (c + 1))
            out[0, rows, :] = r[R, 0:ROWS, :]
            out[1, rows, :] = r[R, ROWS:2 * ROWS, :]
    out += np.asarray(b_o)[None, None, :]
    if _trace:
        return out, res
    return out
